# revision 33
# baseline (speedup 1.0000x reference)
"""Distributed cosine-attention kernel for TRN2 (8 NeuronCores), v2.

Problem (nn_Attention): B=4, N=2048, D_MODEL=1024, HEADS=16, DIM_HEAD=64
  qkv = x @ w_qkv.T + b_qkv ; q,k l2-normalized over head dim;
  attn = softmax(clip-scale * qn @ kn^T); out = (attn @ v) @ w_out.T

Sharding: core c handles batch b=c//2 and global heads hg*8..hg*8+8 (hg=c%2).
Each core computes a partial out^T [D_MODEL, N]; the host sums the two cores
of each batch and transposes.

v2 design (vs v1 baseline at 636us):
  - all matmuls bf16 (FWL weight loads); f32 PSUM accumulate
  - everything SBUF-resident: no DRAM spills for qhat/khat/vhat
  - score PSUM double-buffered ([128,2,512] x2) -> exp per kt, no WAR stall
  - l2norm rsqrt batched per pair: one Ln + one Exp on [4,4,512] (ACT),
    ss tiles staged via DVE copies
  - emission interleaving: QK proj of pair p+1 threaded through attention of
    pair p, out-proj threaded through attention of pair 3, so the PE never
    idles while the ACT exp stream (the ~300us floor) drains
  - V-proj bias via DVE tensor-tensor add on evac (no K=1 bias matmuls)
PSUM budget (8 banks): mm 2 + sg 2x2 + pv 1 + lb 1.
"""
import sys
sys.path.insert(0, "/opt/trn_rl_repo")

from dataclasses import dataclass

import numpy as np

try:
    import ml_dtypes
    ml_bf16 = ml_dtypes.bfloat16
except ImportError:  # pragma: no cover
    ml_bf16 = np.float32

import concourse.bass as bass
import concourse.tile as tile
import concourse.mybir as mybir
from concourse import bacc
from concourse.bass_utils import run_bass_kernel_spmd

F32 = mybir.dt.float32
BF16 = mybir.dt.bfloat16
AF = mybir.ActivationFunctionType

D_MODEL = 1024
HEADS = 16
DIM_HEAD = 64
INNER = HEADS * DIM_HEAD
B = 4
N = 2048
N_CORES = 8
LOG100 = float(np.log(100.0))

_ACT_SET = "natural_log_exp_and_others"
_tables_patched = False


def _patch_act_tables():
    """Make every activation resolve to one table set (it contains ln, exp,
    square, copy, identity) so no ACT_TABLE_LOAD thrash occurs."""
    global _tables_patched
    if _tables_patched:
        return
    orig = bacc.get_activation_tables

    def patched(arch):
        tabs = orig(arch)
        if _ACT_SET in tabs:
            tabs = {k: (v if k == _ACT_SET else set())
                    for k, v in tabs.items()}
        return tabs

    bacc.get_activation_tables = patched
    _tables_patched = True


@dataclass
class Cfg:
    T: int = N
    C: int = D_MODEL
    NH: int = 8
    DH: int = DIM_HEAD
    QB: int = 512
    merge_pairs: tuple = (True, True, True, True)

    @property
    def PAIRS(self):
        return self.NH // 2

    @property
    def CT(self):
        return self.C // 128

    @property
    def KT(self):
        return self.T // 128

    @property
    def NQB(self):
        return self.T // self.QB

    @property
    def VW(self):
        return self.NH * self.DH


def build(cfg: Cfg):
    _patch_act_tables()
    T, C, QB = cfg.T, cfg.C, cfg.QB
    PAIRS, CT, KT, NQB, VW = cfg.PAIRS, cfg.CT, cfg.KT, cfg.NQB, cfg.VW

    nc = bacc.Bacc("TRN2", target_bir_lowering=False, debug=False,
                   enable_asserts=False)

    xT_d = nc.declare_dram_parameter("xT", [C, T], BF16, isOutput=False)
    # weights pre-swizzled host-side so every DMA partition-read is one
    # contiguous descriptor (the in-kernel rearrange patterns cost ~8
    # scattered 256B descriptors per partition and delayed startup)
    wqk_d = nc.declare_dram_parameter("wqk", [128, 2 * PAIRS, CT, 128], BF16,
                                      isOutput=False)
    bqk_d = nc.declare_dram_parameter("bqk", [2 * PAIRS, 128, 1], F32,
                                      isOutput=False)
    wv_d = nc.declare_dram_parameter("wv", [128, CT, VW], BF16,
                                     isOutput=False)
    bv_d = nc.declare_dram_parameter("bv", [1, VW], F32, isOutput=False)
    wo_d = nc.declare_dram_parameter("wo", [128, PAIRS, C], BF16,
                                     isOutput=False)
    # per-head scale constants: [:, 0]=-s_h (exp bias), [:, 1]=ln(s_h)
    scl_d = nc.declare_dram_parameter("scl", [cfg.NH, 2, 1], F32,
                                      isOutput=False)
    # per-pair norm-exp bias col: rows 0,1 (q halves) = 0; rows 32,33
    # (k halves) = ln(s_h) so the k-side normalizer folds in the logit scale
    lnsb_d = nc.declare_dram_parameter("lnsb", [PAIRS, 34, 1], F32,
                                       isOutput=False)
    sel2T_d = nc.declare_dram_parameter("sel2T", [128, 2], BF16,
                                        isOutput=False)
    out_d = nc.declare_dram_parameter("out", [C, T], F32, isOutput=True)

    with tile.TileContext(nc) as tc:
        with (
            tc.tile_pool(name="const", bufs=1) as const,
            tc.tile_pool(name="dram", bufs=1, space="DRAM") as dram,
            tc.tile_pool(name="xt", bufs=1) as xt_pool,
            tc.tile_pool(name="persist", bufs=1) as persist,
            tc.tile_pool(name="wcol", bufs=4) as wcol_pool,
            tc.tile_pool(name="qkw", bufs=8) as qkw_pool,
            tc.tile_pool(name="norm", bufs=2) as norm_sb,
            tc.tile_pool(name="att", bufs=2) as att_sb,
            tc.tile_pool(name="pt", bufs=4) as pt_pool,
            tc.tile_pool(name="ofin", bufs=1) as ofin_pool,
            tc.tile_pool(name="otout", bufs=2) as ot_pool,
            # PSUM budget (8 banks): mm 2 + sg 2x2 + pv 1 + lb 1
            tc.tile_pool(name="psmm", bufs=2, space="PSUM") as ps_mm,
            tc.tile_pool(name="pssg", bufs=2, space="PSUM") as ps_sg,
            tc.tile_pool(name="pspv", bufs=1, space="PSUM") as ps_pv,
        ):
            # ---- DRAM spill tensors (norm factors only, for broadcast) ----
            rq_sp = [dram.tile([34, NQB, QB], F32, tag=f"rqsp{p}",
                               name=f"rqsp{p}") for p in range(PAIRS)]

            # ---- constants ----
            scratch_f = const.tile([128, 64], F32, tag="scratch")
            nc.vector.memset(scratch_f, 1.0)
            ones_bf = const.tile([128, 64], BF16, tag="ones_bf")
            nc.vector.tensor_copy(ones_bf, scratch_f)
            sel2T = const.tile([128, 2], BF16, tag="sel2T")
            nc.sync.dma_start(out=sel2T, in_=sel2T_d.ap())

            nbias_cols = []
            for h in range(cfg.NH):
                col = const.tile([128, 1], F32, tag=f"nb{h}", name=f"nb{h}")
                nc.sync.dma_start(
                    out=col, in_=scl_d.ap()[h, 0:1, :].to_broadcast((128, 1)))
                nbias_cols.append(col)
            lnsb_cols = []
            for p in range(PAIRS):
                col = const.tile([34, 1], F32, tag=f"lnsb{p}",
                                 name=f"lnsb{p}")
                nc.sync.dma_start(out=col, in_=lnsb_d.ap()[p])
                lnsb_cols.append(col)

            bqk_cols = []
            for it in range(2 * PAIRS):
                col = const.tile([128, 1], F32, tag=f"bqk{it}",
                                 name=f"bqk{it}")
                nc.sync.dma_start(out=col, in_=bqk_d.ap()[it])
                bqk_cols.append(col)
            bv_bc = const.tile([128, VW], F32, tag="bv_bc")
            nc.sync.dma_start(out=bv_bc,
                              in_=bv_d.ap().to_broadcast((128, VW)))

            # x tokens first, split in halves so all DMA queues carry x
            # during startup; weight tiles are deferred off the startup path
            xt = []
            for ct in range(CT):
                t = xt_pool.tile([128, T], BF16, tag=f"xt{ct}",
                                 name=f"xt{ct}")
                h = T // 2
                nc.sync.dma_start(
                    out=t[:, 0:h],
                    in_=xT_d.ap()[ct * 128:(ct + 1) * 128, 0:h])
                nc.sync.dma_start(
                    out=t[:, h:T],
                    in_=xT_d.ap()[ct * 128:(ct + 1) * 128, h:T])
                xt.append(t)

            wv_res = const.tile([128, CT, VW], BF16, tag="wv_res")
            wo_res = const.tile([128, PAIRS, C], BF16, tag="wo_res")

            def emit_wv_dma():
                nc.sync.dma_start(out=wv_res, in_=wv_d.ap())

            def emit_wo_dma():
                nc.sync.dma_start(out=wo_res, in_=wo_d.ap())

            # ---- persistent SBUF tensors ----
            qhat = [persist.tile([128, T], BF16, tag=f"qh{p}",
                                 name=f"qh{p}") for p in range(PAIRS)]
            khat = [persist.tile([128, T], BF16, tag=f"kh{p}",
                                 name=f"kh{p}") for p in range(PAIRS)]
            # V, token-partition: [128, KT, VW]; pair p uses cols p*128..
            vv = persist.tile([128, KT, VW], BF16, tag="vv")

            # ================= V projection (16 tt units) =================
            def gen_vproj():
                pend = None
                for tt in range(KT):
                    vps = ps_mm.tile([128, VW], F32, tag="mm",
                                     name=f"vps{tt}")
                    for ct in range(CT):
                        nc.tensor.matmul(vps,
                                         xt[ct][:, tt * 128:(tt + 1) * 128],
                                         wv_res[:, ct, :], start=(ct == 0),
                                         stop=(ct == CT - 1))
                        yield
                    if pend is not None:
                        po, pt_ = pend
                        nc.vector.tensor_add(vv[:, pt_, :], po, bv_bc)
                    pend = (vps, tt)
                    yield
                po, pt_ = pend
                nc.vector.tensor_add(vv[:, pt_, :], po, bv_bc)
                yield

            # ============ QK projection + l2norm (per pair) ============
            # norm staging rows: q halves at partitions 0,1; k halves at
            # 32,33 (ss k-matmul col-tiled to position 32 so every engine
            # access keeps a 32-aligned partition offset)
            def prefetch_wcols(p):
                wcols = []
                for is_k in (0, 1):
                    it = 2 * p + is_k
                    wcol = wcol_pool.tile([128, CT, 128], BF16, tag="wcol",
                                          name=f"wcol{it}")
                    nc.sync.dma_start(out=wcol, in_=wqk_d.ap()[:, it])
                    wcols.append(wcol)
                return wcols

            def gen_qk_pair(p, wcols, dense=False):
                qraw_tiles = {}
                q2_pairs = {tb: [None, None] for tb in range(NQB)}
                stage = norm_sb.tile([34, NQB, QB], F32, tag="stage",
                                     name=f"stage{p}", bufs=1)
                yield

                def flush(is_k, tb, qs):
                    it = 2 * p + is_k
                    uid = f"{it}_{tb}"
                    qraw = qkw_pool.tile([128, QB], BF16, tag="qraw",
                                         name=f"qraw{uid}")
                    nc.vector.tensor_scalar_add(qraw, qs, bqk_cols[it])
                    q2 = qkw_pool.tile([128, QB], BF16, tag="q2",
                                       name=f"q2{uid}", bufs=8)
                    nc.vector.tensor_mul(q2, qraw, qraw)
                    qraw_tiles[(is_k, tb)] = qraw
                    q2_pairs[tb][is_k] = q2

                def emit_ss(tb):
                    ss = ps_mm.tile([34, QB], F32, tag="mm",
                                    name=f"ss{p}_{tb}")
                    nc.tensor.matmul(ss[0:2, :], sel2T, q2_pairs[tb][0],
                                     start=True, stop=True,
                                     tile_position=(0, 0))
                    nc.tensor.matmul(ss[32:34, :], sel2T, q2_pairs[tb][1],
                                     start=True, stop=True,
                                     tile_position=(0, 32),
                                     skip_group_check=True)
                    nc.vector.tensor_copy(stage[:, tb, :], ss)

                if dense:
                    # ct-major: PE starts as soon as xt[0] lands; 4 token
                    # blocks accumulate in the (idle) sg-tag PSUM banks
                    for is_k in (0, 1):
                        qsA = ps_sg.tile([128, 2, QB], F32, tag="sg",
                                         name=f"qsA{p}_{is_k}")
                        qsB = ps_sg.tile([128, 2, QB], F32, tag="sg",
                                         name=f"qsB{p}_{is_k}")
                        accs = [qsA[:, 0, :], qsA[:, 1, :],
                                qsB[:, 0, :], qsB[:, 1, :]]
                        for ct in range(CT):
                            for tb in range(NQB):
                                ts = slice(tb * QB, (tb + 1) * QB)
                                nc.tensor.matmul(accs[tb],
                                                 wcols[is_k][:, ct, :],
                                                 xt[ct][:, ts],
                                                 start=(ct == 0),
                                                 stop=(ct == CT - 1))
                                yield
                        for tb in range(NQB):
                            flush(is_k, tb, accs[tb])
                            yield
                    for tb in range(NQB):
                        emit_ss(tb)
                        yield
                else:
                    pend = None
                    for tb in range(NQB):
                        ts = slice(tb * QB, (tb + 1) * QB)
                        for is_k in (0, 1):
                            qs = ps_mm.tile([128, QB], F32, tag="mm",
                                            name=f"qs{p}_{is_k}_{tb}")
                            for ct in range(CT):
                                nc.tensor.matmul(qs, wcols[is_k][:, ct, :],
                                                 xt[ct][:, ts],
                                                 start=(ct == 0),
                                                 stop=(ct == CT - 1))
                                yield
                            flush(is_k, tb, qs)
                            yield
                            # ss matmuls of the *previous* tb (q2 ready on
                            # DVE well before the PE reaches these matmuls)
                            if pend is not None:
                                emit_ss(pend)
                                pend = None
                        pend = tb
                        yield
                    emit_ss(pend)
                    yield
                # batched rsqrt via Ln + Exp (stays in one ACT table set):
                # rq = exp(-0.5*ln(ss) + lnsb) = s_h * ss^-0.5 (k rows)
                lss = norm_sb.tile([34, NQB, QB], F32, tag="lss",
                                   name=f"lss{p}", bufs=1)
                nc.scalar.activation(lss, stage, AF.Ln)
                rq = norm_sb.tile([34, NQB, QB], F32, tag="rq",
                                  name=f"rq{p}", bufs=1)
                nc.scalar.activation(rq, lss, AF.Exp, scale=-0.5,
                                     bias=lnsb_cols[p])
                nc.sync.dma_start(out=rq_sp[p], in_=rq)
                yield
                for is_k in (0, 1):
                    for tb in range(NQB):
                        ts = slice(tb * QB, (tb + 1) * QB)
                        uid = f"{p}_{is_k}_{tb}"
                        r0 = 32 * is_k
                        rqbc = qkw_pool.tile([128, QB], F32, tag="rqbc",
                                             name=f"rqbc{uid}", bufs=2)
                        nc.sync.dma_start(
                            out=rqbc[0:64, :],
                            in_=rq_sp[p][r0:r0 + 1, tb, :]
                            .to_broadcast((64, QB)))
                        nc.sync.dma_start(
                            out=rqbc[64:128, :],
                            in_=rq_sp[p][r0 + 1:r0 + 2, tb, :]
                            .to_broadcast((64, QB)))
                        dst = khat[p] if is_k else qhat[p]
                        nc.vector.tensor_mul(dst[:, ts],
                                             qraw_tiles[(is_k, tb)], rqbc)
                        yield

            # ================= out projection (per qb) =================
            o_fin = {}

            def gen_outproj_qb(qb):
                pend = None
                for cb in range(CT):
                    op = ps_mm.tile([128, QB], F32, tag="mm",
                                    name=f"op{qb}_{cb}")
                    for p in range(PAIRS):
                        nc.tensor.matmul(op,
                                         wo_res[:, p,
                                                cb * 128:(cb + 1) * 128],
                                         o_fin[(p, qb)][:],
                                         start=(p == 0),
                                         stop=(p == PAIRS - 1))
                        yield
                    if pend is not None:
                        po, pcb = pend
                        ot = ot_pool.tile([128, QB], F32, tag="ot",
                                          name=f"ot{qb}_{pcb}")
                        nc.vector.tensor_copy(ot, po)
                        nc.sync.dma_start(
                            out=out_d.ap()[pcb * 128:(pcb + 1) * 128,
                                           qb * QB:(qb + 1) * QB], in_=ot)
                    pend = (op, cb)
                    yield
                po, pcb = pend
                ot = ot_pool.tile([128, QB], F32, tag="ot",
                                  name=f"ot{qb}_{pcb}")
                nc.vector.tensor_copy(ot, po)
                nc.sync.dma_start(
                    out=out_d.ap()[pcb * 128:(pcb + 1) * 128,
                                   qb * QB:(qb + 1) * QB], in_=ot)
                yield

            # ================= attention (per pair) =================
            class Filler:
                def __init__(self):
                    self.gens = []  # (tag, generator) FIFO

                def add(self, g, tag=None):
                    self.gens.append((tag, g))

                def pop(self, n):
                    while n > 0 and self.gens:
                        try:
                            next(self.gens[0][1])
                            n -= 1
                        except StopIteration:
                            self.gens.pop(0)

                def drain_tag(self, tag):
                    """Fully emit every queued generator up to and including
                    the one labelled `tag`, leaving later ones queued."""
                    while self.gens and not (self.gens and
                                             self.gens[0][0] is not None and
                                             self.gens[0][0] > tag):
                        try:
                            next(self.gens[0][1])
                        except StopIteration:
                            self.gens.pop(0)

                def drain(self):
                    while self.gens:
                        try:
                            next(self.gens[0][1])
                        except StopIteration:
                            self.gens.pop(0)

            def emit_att_pair(p, fill, budget):
                """budget: filler thunks to interleave per kt group."""
                vsl = slice(p * 128, (p + 1) * 128)
                kk, qq = khat[p], qhat[p]
                n_groups = NQB * KT
                gi = 0
                emitted = 0.0
                for qb in range(NQB):
                    qsl = slice(qb * QB, (qb + 1) * QB)
                    pv = ps_pv.tile([128, QB], F32, tag="pv",
                                    name=f"pv{p}_{qb}")
                    lb = ps_pv.tile([128, QB], F32, tag="lb",
                                    name=f"lb{p}_{qb}")
                    pend = None

                    def emit_pvlb(kt, ptile, pv=pv, lb=lb):
                        first = kt == 0
                        last = kt == KT - 1
                        v0 = vv[:, kt, vsl][:, 0:64]
                        v1 = vv[:, kt, vsl][:, 64:128]
                        nc.tensor.matmul(pv[0:64, :], v0, ptile[:, 0, :],
                                         start=first, stop=last,
                                         tile_position=(0, 0))
                        nc.tensor.matmul(pv[64:128, :], v1, ptile[:, 1, :],
                                         start=first, stop=last,
                                         tile_position=(0, 64),
                                         skip_group_check=True)
                        nc.tensor.matmul(lb[0:64, :], ones_bf, ptile[:, 0, :],
                                         start=first, stop=last,
                                         tile_position=(0, 0))
                        nc.tensor.matmul(lb[64:128, :], ones_bf,
                                         ptile[:, 1, :],
                                         start=first, stop=last,
                                         tile_position=(0, 64),
                                         skip_group_check=True)

                    for kt in range(KT):
                        if gi < 6:
                            # pair start: the first scores stall on the sg
                            # WAR until the prior pair's exp queue drains;
                            # prefetched fillers emitted before them keep
                            # the in-order PE stream busy through the bubble
                            fill.pop(2)
                            emitted += 2
                        ksl = slice(kt * 128, (kt + 1) * 128)
                        sg = ps_sg.tile([128, 2, QB], F32, tag="sg",
                                        name=f"sg{p}_{qb}_{kt}")
                        nc.tensor.matmul(sg[:, 0, :], kk[0:64, ksl],
                                         qq[0:64, qsl], start=True,
                                         stop=True)
                        nc.tensor.matmul(sg[:, 1, :], kk[64:128, ksl],
                                         qq[64:128, qsl], start=True,
                                         stop=True)
                        ptile = pt_pool.tile([128, 2, QB], BF16, tag="pt",
                                             name=f"pt{p}_{qb}_{kt}")
                        if cfg.merge_pairs[p]:
                            nc.scalar.activation(ptile, sg, AF.Exp,
                                                 bias=nbias_cols[2 * p][:])
                        else:
                            nc.scalar.activation(ptile[:, 0], sg[:, 0],
                                                 AF.Exp,
                                                 bias=nbias_cols[2 * p][:])
                            nc.scalar.activation(
                                ptile[:, 1], sg[:, 1], AF.Exp,
                                bias=nbias_cols[2 * p + 1][:])
                        if pend is not None:
                            emit_pvlb(*pend)
                        pend = (kt, ptile)
                        gi += 1
                        want = budget * gi
                        k = int(want - emitted)
                        if k > 0:
                            fill.pop(k)
                            emitted += k
                    emit_pvlb(*pend)

                    rl = att_sb.tile([128, QB], F32, tag="rl",
                                     name=f"rl{p}_{qb}")
                    nc.vector.reciprocal_approx_fast(out=rl, in_=lb)
                    of = ofin_pool.tile([128, QB], BF16, tag=f"of{p}_{qb}",
                                        name=f"of{p}_{qb}")
                    nc.vector.tensor_mul(of, pv, rl)
                    o_fin[(p, qb)] = of
                    yield qb

            # ======== top-level schedule ========
            fill = Filler()
            # pair 0 QK proj + norm, emitted densely (ct-major)
            wc0 = prefetch_wcols(0)
            for _ in gen_qk_pair(0, wc0, dense=True):
                pass
            emit_wv_dma()
            wc_next = prefetch_wcols(1)
            # V projection, emitted densely
            for _ in gen_vproj():
                pass
            # attention p with QK proj of p+1 interleaved; weight tiles for
            # pair p+2 prefetched one pair early so boundary fillers are
            # runnable the moment they pop
            for p in range(PAIRS):
                if p == 2:
                    emit_wo_dma()
                if p + 1 < PAIRS:
                    fill.add(gen_qk_pair(p + 1, wc_next), tag=p + 1)
                    if p + 2 < PAIRS:
                        wc_next = prefetch_wcols(p + 2)
                    budget = 2.0
                else:
                    budget = 3.2
                for done_qb in emit_att_pair(p, fill, budget):
                    if p == PAIRS - 1:
                        # all pairs done for this qb -> out proj becomes
                        # legal filler
                        fill.add(gen_outproj_qb(done_qb))
                fill.drain()

    nc.compile()
    return nc


# ======================= host-side sharding =======================

def shard_inputs(x, w_qkv, b_qkv, w_out, logit_scale):
    x = np.ascontiguousarray(np.asarray(x, dtype=np.float32))
    w_qkv = np.asarray(w_qkv, dtype=np.float32)
    b_qkv = np.asarray(b_qkv, dtype=np.float32)
    w_out = np.asarray(w_out, dtype=np.float32)
    ls = np.asarray(logit_scale, dtype=np.float32).reshape(-1)
    s_all = np.exp(np.minimum(ls, LOG100)).astype(np.float32)

    Wq = w_qkv[0 * INNER:1 * INNER]
    Wk = w_qkv[1 * INNER:2 * INNER]
    Wv = w_qkv[2 * INNER:3 * INNER]
    bq = b_qkv[0 * INNER:1 * INNER]
    bk = b_qkv[1 * INNER:2 * INNER]
    bv = b_qkv[2 * INNER:3 * INNER]

    xT = [np.ascontiguousarray(x[b].T.astype(ml_bf16)) for b in range(B)]

    per_hg = {}
    merge = [True] * 4
    for hg in range(2):
        heads = list(range(hg * 8, hg * 8 + 8))
        rows, brows = [], []
        lnsb = np.zeros((4, 34, 1), dtype=np.float32)
        for p in range(4):
            g0, g1 = heads[2 * p], heads[2 * p + 1]
            if s_all[g0] != s_all[g1]:
                merge[p] = False
            rows += [Wq[g0 * 64:(g0 + 1) * 64], Wq[g1 * 64:(g1 + 1) * 64],
                     Wk[g0 * 64:(g0 + 1) * 64], Wk[g1 * 64:(g1 + 1) * 64]]
            brows += [bq[g0 * 64:(g0 + 1) * 64], bq[g1 * 64:(g1 + 1) * 64],
                      bk[g0 * 64:(g0 + 1) * 64], bk[g1 * 64:(g1 + 1) * 64]]
            lnsb[p, 32, 0] = np.log(s_all[g0])
            lnsb[p, 33, 0] = np.log(s_all[g1])
        # wqk: [c, it*128+i] -> [pp, it, ct, i] with c = ct*128+pp, so each
        # DMA partition-read is contiguous
        wqkT = np.concatenate(rows, axis=0).T.astype(ml_bf16)  # [C, 1024]
        wqk = np.ascontiguousarray(
            wqkT.reshape(8, 128, 8, 128).transpose(1, 2, 0, 3))
        bqk = np.ascontiguousarray(
            np.concatenate(brows, axis=0)).reshape(8, 128, 1)
        vsl = slice(hg * 512, (hg + 1) * 512)
        wvT = Wv[vsl].T.astype(ml_bf16)                        # [C, 512]
        wv = np.ascontiguousarray(wvT.reshape(8, 128, 512).transpose(1, 0, 2))
        bvs = np.ascontiguousarray(bv[vsl].reshape(1, 512))
        woT = w_out[:, vsl].T.astype(ml_bf16)                  # [512, C]
        wo = np.ascontiguousarray(woT.reshape(4, 128, 1024).transpose(1, 0, 2))
        scl = np.stack([-s_all[heads], np.log(s_all[heads])],
                       axis=1).astype(np.float32).reshape(8, 2, 1)
        per_hg[hg] = dict(wqk=wqk, bqk=bqk, wv=wv, bv=bvs, wo=wo,
                          scl=scl, lnsb=lnsb)

    sel2 = np.zeros((2, 128), dtype=np.float32)
    sel2[0, 0:64] = 1.0
    sel2[1, 64:128] = 1.0
    sel2T = np.ascontiguousarray(sel2.T.astype(ml_bf16))
    in_maps = []
    for c in range(N_CORES):
        b, hg = c // 2, c % 2
        m = dict(per_hg[hg])
        m["xT"] = xT[b]
        m["sel2T"] = sel2T
        in_maps.append(m)
    return in_maps, tuple(merge)


_NC_CACHE = {}
TRACE = False
LAST_RESULT = None


def kernel(x, w_qkv, b_qkv, w_out, logit_scale):
    global LAST_RESULT
    in_maps, merge_pairs = shard_inputs(x, w_qkv, b_qkv, w_out, logit_scale)
    cfg = Cfg(merge_pairs=merge_pairs)
    if merge_pairs not in _NC_CACHE:
        _NC_CACHE[merge_pairs] = build(cfg)
    nc = _NC_CACHE[merge_pairs]
    res = run_bass_kernel_spmd(nc, in_maps, core_ids=list(range(N_CORES)),
                               trace=TRACE)
    LAST_RESULT = res
    outs = [res.results[c]["out"] for c in range(N_CORES)]
    full = np.empty((B, N, D_MODEL), dtype=np.float32)
    for b in range(B):
        full[b] = (outs[2 * b] + outs[2 * b + 1]).T
    return full
